# revision 20
# baseline (speedup 1.0000x reference)
"""2-layer GCN (PyG GCNConv x2 + leaky_relu) on 8 Trainium2 NeuronCores.

v2 strategy (dst-partitioned, gather-free, For_i hardware loops):
  - Nodes partitioned 128-ALIGNED across 8 cores: core c owns padded ids
    [c*6272, c*6272+6250); padded table has NPAD=50176 rows (zeros in pads).
  - Normalization folded: table rows pre-scaled by dis[src]; dis[dst] applied
    post-aggregation (ACT scale). Self-loops NOT in the edge stream: handled
    by one identity matmul per dst tile reading the core's own rows
    (xown for layer 1, `part` for layer 2) — contributes dis_d^2 * x_d.
  - Edge routing has NO per-edge DMA. Two phases through a DRAM scratch M:
    Phase 1 (For_i over 392 src blocks): one-hot Sel (DVE is_equal vs iota)
      selects/duplicates rows of X_b [128,64] into bucket slots via PE
      matmul; 7 chunks/block -> stage [128,7*64] -> one contiguous DMA to
      M block region [896 rows, 64]. Block region row m holds slot
      (p=m//7, c=m%7); bucket (b,t) occupies rows m = t*16 + r (r<16).
    Phase 2 (For_i over 49 dst tiles): one strided DMA reads rows
      [16t,16t+16) of every block -> msb [128, 49*64] (slot q=b*16+r at
      partition q//49, col q%49); 49 one-hot scatter matmuls accumulate
      agg[128dst,64] in PSUM; + identity matmul (self loop) (+ rank-1
      disinv x b2 term closing layer 2's group).
  - Layer-1 tail per tile: ACT(dis) -> PE transpose -> W1 -> Lrelu+b1 ->
    W2 -> ACT(dis) -> part. One AllGather builds the layer-2 table.
  - Bucket capacity R=16 (R grows by 16s if the graph needs it).

Self-contained: hardcodes shapes; compiles on first call keyed by edge hash.
"""

import os
import hashlib
import sys

import numpy as np

sys.path.insert(0, "/opt/trn_rl_repo")

# ---- problem constants ----
N, E = 50000, 800000
DIN, DH, DOUT = 64, 128, 64
P_CORES = 8
NP = N // P_CORES            # 6250 real nodes per core
NT = 49                      # dst tiles per core
NPP = NT * 128               # 6272 padded rows per core
NPAD = P_CORES * NPP         # 50176 padded table rows
NB = NPAD // 128             # 392 real src blocks (global)
NBM = 512                    # M-scratch block slots (pad => CPT = 4R exactly)
PAD = 200.0                  # one-hot miss value
NEG_SLOPE = 0.01


def _prep(edge_index: np.ndarray):
    src = np.asarray(edge_index[0], dtype=np.int64)
    dst = np.asarray(edge_index[1], dtype=np.int64)

    deg = (np.bincount(dst, minlength=N) + 1).astype(np.float32)
    dis = (1.0 / np.sqrt(deg)).astype(np.float32)

    pid_src = (src // NP) * NPP + (src % NP)      # padded id of src
    core = dst // NP
    tloc = (dst // NP) * 0 + (dst % NP)           # local dst 0..NP-1
    b_all = pid_src // 128                        # src block 0..NB-1
    t_all = tloc // 128                           # dst tile 0..NT-1
    srclo_all = pid_src % 128
    dstlo_all = tloc % 128

    # bucket ranks per (core, b, t)
    key = (core * NB + b_all) * NT + t_all
    order = np.argsort(key, kind="stable")
    ks = key[order]
    # rank within equal keys
    first = np.ones(len(ks), dtype=bool)
    first[1:] = ks[1:] != ks[:-1]
    starts = np.flatnonzero(first)
    run_id = np.cumsum(first) - 1
    r_sorted = np.arange(len(ks)) - starts[run_id]
    rmax = int(r_sorted.max()) + 1 if len(ks) else 1
    R = max(16, rmax)                             # bucket capacity
    SPB = ((NT * R + 127) // 128) * 128           # slots per block region
    CPB = SPB // 128                              # phase-1 chunks per block
    CPT = (NBM * R) // 128                        # phase-2 chunks per tile

    # srcloc: row vector of src-lo per slot, j = c*128 + p for slot at
    # physical row m = p*CPB + c (transposed one-hot built on device via
    # rank-1 PE broadcast + tensor_tensor is_equal)
    srcloc = np.full((P_CORES, 1, NBM * SPB), PAD, dtype=np.float32)
    dstloc = np.full((P_CORES, 128, NT * CPT), PAD, dtype=np.float32)

    co = core[order]
    bo = b_all[order]
    to = t_all[order]
    so = srclo_all[order]
    do = dstlo_all[order]
    # phase 1: block-region row m = t*R + r at (p=m//CPB, c=m%CPB)
    m = to * R + r_sorted
    p1 = m // CPB
    c1 = m % CPB
    srcloc[co, 0, bo * SPB + c1 * 128 + p1] = so.astype(np.float32)
    # phase 2: tile stream position q = b*R + r at (p=q//CPT, j=q%CPT)
    q = bo * R + r_sorted
    p2 = q // CPT
    j2 = q % CPT
    dstloc[co, p2, to * CPT + j2] = do.astype(np.float32)

    dis_t = np.zeros((P_CORES, 128, NT), dtype=np.float32)
    dinv = np.zeros((P_CORES, 1, NPP), dtype=np.float32)
    for c in range(P_CORES):
        d = dis[c * NP:(c + 1) * NP]
        pad = np.zeros(NPP, dtype=np.float32)
        pad[:NP] = d
        dis_t[c] = pad.reshape(NT, 128).T
        ipad = np.zeros(NPP, dtype=np.float32)
        ipad[:NP] = 1.0 / d
        dinv[c, 0] = ipad

    return dict(dis=dis, R=R, SPB=SPB, CPB=CPB, CPT=CPT,
                srcloc=srcloc, dstloc=dstloc, dis_t=dis_t, dinv=dinv)


# ---------------------------------------------------------------------------
# Bass kernel
# ---------------------------------------------------------------------------

def _build_nc(prep):
    import concourse.bass as bass
    import concourse.bacc as bacc
    import concourse.tile as tile
    from concourse import mybir

    f32 = mybir.dt.float32
    AF = mybir.ActivationFunctionType
    ALU = mybir.AluOpType
    ds = bass.ds

    R, CPB, CPT, SPB = prep["R"], prep["CPB"], prep["CPT"], prep["SPB"]

    nc = bacc.Bacc(
        "TRN2", target_bir_lowering=False, debug=False,
        enable_asserts=False, num_devices=P_CORES,
    )

    H2 = SPB // 2
    xt_d = nc.dram_tensor("xt", [NPAD, DIN], f32, kind="ExternalInput")
    xown_d = nc.dram_tensor("xown", [NPP, DIN], f32, kind="ExternalInput")
    srcloc_d = nc.dram_tensor("srcloc", [1, NBM * SPB], f32,
                              kind="ExternalInput")
    ones1_d = nc.dram_tensor("ones1", [1, 128], f32, kind="ExternalInput")
    iotat_d = nc.dram_tensor("iotat", [128, H2], f32, kind="ExternalInput")
    dstloc_d = nc.dram_tensor("dstloc", [128, NT * CPT], f32,
                              kind="ExternalInput")
    dis_d = nc.dram_tensor("dis_t", [128, NT], f32, kind="ExternalInput")
    dinv_d = nc.dram_tensor("dinv", [1, NPP], f32, kind="ExternalInput")
    w1_d = nc.dram_tensor("w1", [DIN, DH], f32, kind="ExternalInput")
    w2_d = nc.dram_tensor("w2", [DH, DOUT], f32, kind="ExternalInput")
    b1_d = nc.dram_tensor("b1", [DH, 1], f32, kind="ExternalInput")
    b2_d = nc.dram_tensor("b2r", [1, DOUT], f32, kind="ExternalInput")
    iota_d = nc.dram_tensor("iota", [128, 128], f32, kind="ExternalInput")
    ident_d = nc.dram_tensor("ident", [128, 128], f32, kind="ExternalInput")
    out_d = nc.dram_tensor("outp", [NPP, DOUT], f32, kind="ExternalOutput")

    with tile.TileContext(nc) as tc:
        with (
            tc.tile_pool(name="const", bufs=1) as constp,
            tc.tile_pool(name="xb", bufs=3) as xpool,
            tc.tile_pool(name="stg", bufs=3) as stpool,
            tc.tile_pool(name="sl", bufs=3) as slpool,
            tc.tile_pool(name="sp", bufs=4) as spool,
            tc.tile_pool(name="msb", bufs=2) as mpool,
            tc.tile_pool(name="wk", bufs=2) as work,
            tc.tile_pool(name="p1", bufs=2, space="PSUM") as p1pool,
            tc.tile_pool(name="psel", bufs=1, space="PSUM") as pselp,
            tc.tile_pool(name="pagg", bufs=2, space="PSUM") as pagg,
            tc.tile_pool(name="ptr", bufs=1, space="PSUM") as ptr,
            tc.tile_pool(name="pg1", bufs=1, space="PSUM") as pg1,
            tc.tile_pool(name="pg2", bufs=1, space="PSUM") as pg2,
            tc.tile_pool(name="dram", bufs=1, space="DRAM") as dram,
        ):
            iota_sb = constp.tile([128, 128], f32)
            ident_sb = constp.tile([128, 128], f32)
            ones1_sb = constp.tile([1, 128], f32)
            iotat_sb = constp.tile([128, H2], f32)
            w1_sb = constp.tile([DIN, DH], f32)
            w2_sb = constp.tile([DH, DOUT], f32)
            b1_sb = constp.tile([DH, 1], f32)
            b2_sb = constp.tile([1, DOUT], f32)
            for sb, dr in [(iota_sb, iota_d), (ident_sb, ident_d),
                           (ones1_sb, ones1_d), (iotat_sb, iotat_d),
                           (w1_sb, w1_d), (w2_sb, w2_d),
                           (b1_sb, b1_d), (b2_sb, b2_d)]:
                nc.sync.dma_start(sb[:], dr[:])

            # M scratch lives across repeats; zero the pad-block regions
            # once (phase 2 reads them; dstloc=PAD keeps them out of sums,
            # but they must be finite).
            M_d = dram.tile([NBM, SPB * DIN], f32, tag="M", bufs=1)
            zt = work.tile([128, SPB * DIN // 128], f32, tag="zt")
            nc.gpsimd.memset(zt[:], 0.0)
            for b in range(NB, NBM):
                nc.sync.dma_start(M_d[b:b + 1, :], zt[:])

            for _rep in range(int(os.environ.get("GCN_REPEAT", "1"))):
                part = dram.tile([NPP, DOUT], f32, tag="part", bufs=2)
                table = dram.tile([NPAD, DOUT], f32, addr_space="Shared",
                                  tag="table", bufs=2)

                for lidx in range(2):
                    src_d = xt_d if lidx == 0 else table
                    own_d = xown_d if lidx == 0 else part
                    # ---- phase 1: route src blocks into bucket slots ----
                    with tc.For_i(0, NB) as i:
                        xb = xpool.tile([128, DIN], f32, tag="xb")
                        nc.sync.dma_start(xb[:], src_d[ds(i * 128, 128), :])
                        slst = slpool.tile([1, SPB], f32, tag="slst")
                        nc.sync.dma_start(
                            slst[:], srcloc_d[:, ds(i * SPB, SPB)])
                        # transposed one-hot: sel[s, j] = (s == srclo(slot j))
                        sel = spool.tile([128, SPB], f32, tag="sel")
                        for h in range(2):
                            pr = pselp.tile([128, H2], f32, tag="pr")
                            nc.tensor.matmul(
                                pr[:], ones1_sb[:],
                                slst[:, h * H2:(h + 1) * H2],
                                start=True, stop=True)
                            nc.vector.tensor_tensor(
                                sel[:, h * H2:(h + 1) * H2], iotat_sb[:],
                                pr[:], op=ALU.is_equal)
                        ps = p1pool.tile([128, CPB * DIN], f32, tag="ps")
                        for c in range(CPB):
                            nc.tensor.matmul(
                                ps[:, c * DIN:(c + 1) * DIN],
                                sel[:, c * 128:(c + 1) * 128], xb[:],
                                start=True, stop=True)
                        stage = stpool.tile([128, CPB * DIN], f32,
                                            tag="stage")
                        nc.scalar.activation(stage[:], ps[:], AF.Copy,
                                             bias=0.0)
                        nc.sync.dma_start(M_d[ds(i, 1), :], stage[:])
                    # ---- phase 2: per dst tile aggregate + layer tail ----
                    with tc.For_i(0, NT) as i:
                        msb = mpool.tile([128, CPT * DIN], f32, tag="msb")
                        nc.sync.dma_start(
                            msb[:],
                            M_d[:, ds(i * (R * DIN), R * DIN)].rearrange(
                                "b (r f) -> b r f", f=DIN))
                        xdg = xpool.tile([128, DIN], f32, tag="xdg")
                        nc.sync.dma_start(xdg[:], own_d[ds(i * 128, 128), :])
                        dlst = slpool.tile([128, CPT], f32, tag="dlst")
                        nc.sync.dma_start(
                            dlst[:], dstloc_d[:, ds(i * CPT, CPT)])
                        dcol = slpool.tile([128, 1], f32, tag="dcol")
                        nc.sync.dma_start(dcol[:], dis_d[:, ds(i, 1)])
                        agg = pagg.tile([128, DIN], f32, tag="agg")
                        for j in range(CPT):
                            s_t = spool.tile([128, 128], f32, tag="s2")
                            nc.vector.tensor_scalar(
                                s_t[:], iota_sb[:], dlst[:, j:j + 1], None,
                                ALU.is_equal)
                            nc.tensor.matmul(
                                agg[:], s_t[:],
                                msb[:, j * DIN:(j + 1) * DIN],
                                start=(j == 0), stop=False)
                        nc.tensor.matmul(agg[:], ident_sb[:], xdg[:],
                                         start=False, stop=(lidx == 0))
                        if lidx == 0:
                            a_sb = work.tile([128, DIN], f32, tag="a_sb")
                            nc.scalar.activation(a_sb[:], agg[:], AF.Copy,
                                                 bias=0.0,
                                                 scale=dcol[:, 0:1])
                            aT_p = ptr.tile([DIN, 128], f32, tag="aT")
                            nc.tensor.transpose(aT_p[:], a_sb[:], ident_sb[:])
                            aT_sb = work.tile([DIN, 128], f32, tag="aT_sb")
                            nc.scalar.activation(aT_sb[:], aT_p[:], AF.Copy,
                                                 bias=0.0)
                            x1_p = pg1.tile([DH, 128], f32, tag="x1")
                            nc.tensor.matmul(x1_p[:], w1_sb[:], aT_sb[:],
                                             start=True, stop=True)
                            x1_sb = work.tile([DH, 128], f32, tag="x1_sb")
                            nc.scalar.activation(x1_sb[:], x1_p[:], AF.Lrelu,
                                                 bias=b1_sb[:, 0:1],
                                                 alpha=NEG_SLOPE)
                            x2_p = pg2.tile([128, DOUT], f32, tag="x2")
                            nc.tensor.matmul(x2_p[:], x1_sb[:], w2_sb[:],
                                             start=True, stop=True)
                            x2_sb = work.tile([128, DOUT], f32, tag="x2_sb")
                            nc.scalar.activation(x2_sb[:], x2_p[:], AF.Copy,
                                                 bias=0.0,
                                                 scale=dcol[:, 0:1])
                            nc.sync.dma_start(
                                part[ds(i * 128, 128), :], x2_sb[:])
                        else:
                            div = slpool.tile([1, 128], f32, tag="div")
                            nc.sync.dma_start(
                                div[:], dinv_d[:, ds(i * 128, 128)])
                            nc.tensor.matmul(agg[:], div[:], b2_sb[:],
                                             start=False, stop=True)
                            o_sb = work.tile([128, DOUT], f32, tag="o_sb")
                            nc.scalar.activation(o_sb[:], agg[:], AF.Lrelu,
                                                 bias=0.0,
                                                 scale=dcol[:, 0:1],
                                                 alpha=NEG_SLOPE)
                            nc.sync.dma_start(
                                out_d[ds(i * 128, 128), :], o_sb[:])
                    if lidx == 0:
                        if os.environ.get("GCN_NOAG", "0") == "1":
                            # timing ablation: replace exchange with a local
                            # copy of this core's part (results are wrong)
                            nc.sync.dma_start(
                                table[0:NPP, :], part[:, :])
                        else:
                            nc.gpsimd.collective_compute(
                                "AllGather", mybir.AluOpType.bypass,
                                replica_groups=[list(range(P_CORES))],
                                ins=[part.opt()], outs=[table.opt()],
                            )

    nc.compile()
    return nc


def _make_in_maps(inputs, W1, b1, W2, b2, prep):
    dis = prep["dis"]
    xt = np.zeros((NPAD, DIN), dtype=np.float32)
    x32 = np.asarray(inputs, np.float32)
    for c in range(P_CORES):
        xt[c * NPP:c * NPP + NP] = (x32[c * NP:(c + 1) * NP]
                                    * dis[c * NP:(c + 1) * NP, None])
    iota = np.tile(np.arange(128, dtype=np.float32), (128, 1))
    ident = np.eye(128, dtype=np.float32)
    in_maps = []
    for c in range(P_CORES):
        in_maps.append({
            "xt": xt,
            "xown": xt[c * NPP:(c + 1) * NPP],
            "srcloc": prep["srcloc"][c],
            "dstloc": prep["dstloc"][c],
            "dis_t": prep["dis_t"][c],
            "dinv": prep["dinv"][c],
            "w1": np.asarray(W1, np.float32),
            "w2": np.asarray(W2, np.float32),
            "b1": np.asarray(b1, np.float32).reshape(DH, 1),
            "b2r": np.asarray(b2, np.float32).reshape(1, DOUT),
            "iota": iota,
            "ident": ident,
            "ones1": np.ones((1, 128), dtype=np.float32),
            "iotat": np.tile(
                np.arange(128, dtype=np.float32)[:, None],
                (1, prep["SPB"] // 2)),
        })
    return in_maps


_CACHE = {}


def kernel(inputs, edge_index, W1, b1, W2, b2, _trace=False, _results_box=None):
    from concourse.bass_utils import run_bass_kernel_spmd

    edge_index = np.asarray(edge_index)
    key = hashlib.sha1(edge_index.tobytes()).hexdigest()
    key += ":r%s:n%s" % (os.environ.get("GCN_REPEAT", "1"),
                         os.environ.get("GCN_NOAG", "0"))
    if key not in _CACHE:
        prep = _prep(edge_index)
        nc = _build_nc(prep)
        _CACHE[key] = (prep, nc)
    prep, nc = _CACHE[key]
    in_maps = _make_in_maps(inputs, W1, b1, W2, b2, prep)
    res = run_bass_kernel_spmd(
        nc, in_maps, core_ids=list(range(P_CORES)), trace=_trace,
    )
    if _results_box is not None:
        _results_box.append(res)
    out = np.concatenate(
        [res.results[c]["outp"][:NP] for c in range(P_CORES)], axis=0
    )
    return out.astype(np.float32)


# revision 22
# speedup vs baseline: 16.2894x; 16.2894x over previous
"""2-layer GCN (PyG GCNConv x2 + leaky_relu) on 8 Trainium2 NeuronCores.

v2 strategy (dst-partitioned, gather-free, For_i hardware loops):
  - Nodes partitioned 128-ALIGNED across 8 cores: core c owns padded ids
    [c*6272, c*6272+6250); padded table has NPAD=50176 rows (zeros in pads).
  - Normalization folded: table rows pre-scaled by dis[src]; dis[dst] applied
    post-aggregation (ACT scale). Self-loops NOT in the edge stream: handled
    by one identity matmul per dst tile reading the core's own rows
    (xown for layer 1, `part` for layer 2) — contributes dis_d^2 * x_d.
  - Edge routing has NO per-edge DMA. Two phases through a DRAM scratch M:
    Phase 1 (For_i over 392 src blocks): one-hot Sel (DVE is_equal vs iota)
      selects/duplicates rows of X_b [128,64] into bucket slots via PE
      matmul; 7 chunks/block -> stage [128,7*64] -> one contiguous DMA to
      M block region [896 rows, 64]. Block region row m holds slot
      (p=m//7, c=m%7); bucket (b,t) occupies rows m = t*16 + r (r<16).
    Phase 2 (For_i over 49 dst tiles): one strided DMA reads rows
      [16t,16t+16) of every block -> msb [128, 49*64] (slot q=b*16+r at
      partition q//49, col q%49); 49 one-hot scatter matmuls accumulate
      agg[128dst,64] in PSUM; + identity matmul (self loop) (+ rank-1
      disinv x b2 term closing layer 2's group).
  - Layer-1 tail per tile: ACT(dis) -> PE transpose -> W1 -> Lrelu+b1 ->
    W2 -> ACT(dis) -> part. One AllGather builds the layer-2 table.
  - Bucket capacity R=16 (R grows by 16s if the graph needs it).

Self-contained: hardcodes shapes; compiles on first call keyed by edge hash.
"""

import os
import hashlib
import sys

import numpy as np

sys.path.insert(0, "/opt/trn_rl_repo")

# ---- problem constants ----
N, E = 50000, 800000
DIN, DH, DOUT = 64, 128, 64
P_CORES = 8
NP = N // P_CORES            # 6250 real nodes per core
NT = 49                      # dst tiles per core
NPP = NT * 128               # 6272 padded rows per core
NPAD = P_CORES * NPP         # 50176 padded table rows
NB = NPAD // 128             # 392 real src blocks (global)
NBM = 512                    # M-scratch block slots (pad => CPT = 4R exactly)
PAD = 200.0                  # one-hot miss value
NEG_SLOPE = 0.01


def _prep(edge_index: np.ndarray):
    src = np.asarray(edge_index[0], dtype=np.int64)
    dst = np.asarray(edge_index[1], dtype=np.int64)

    deg = (np.bincount(dst, minlength=N) + 1).astype(np.float32)
    dis = (1.0 / np.sqrt(deg)).astype(np.float32)

    pid_src = (src // NP) * NPP + (src % NP)      # padded id of src
    core = dst // NP
    tloc = (dst // NP) * 0 + (dst % NP)           # local dst 0..NP-1
    b_all = pid_src // 128                        # src block 0..NB-1
    t_all = tloc // 128                           # dst tile 0..NT-1
    srclo_all = pid_src % 128
    dstlo_all = tloc % 128

    # bucket ranks per (core, b, t)
    key = (core * NB + b_all) * NT + t_all
    order = np.argsort(key, kind="stable")
    ks = key[order]
    # rank within equal keys
    first = np.ones(len(ks), dtype=bool)
    first[1:] = ks[1:] != ks[:-1]
    starts = np.flatnonzero(first)
    run_id = np.cumsum(first) - 1
    r_sorted = np.arange(len(ks)) - starts[run_id]
    rmax = int(r_sorted.max()) + 1 if len(ks) else 1
    R = max(16, rmax)                             # bucket capacity
    SPB = ((NT * R + 127) // 128) * 128           # slots per block region
    CPB = SPB // 128                              # phase-1 chunks per block
    CPT = (NBM * R) // 128                        # phase-2 chunks per tile

    # srcloc: row vector of src-lo per slot, j = c*128 + p for slot at
    # physical row m = p*CPB + c (transposed one-hot built on device via
    # rank-1 PE broadcast + tensor_tensor is_equal)
    srcloc = np.full((P_CORES, 1, NBM * SPB), PAD, dtype=np.float32)
    dstloc = np.full((P_CORES, 128, NT * CPT), PAD, dtype=np.float32)

    co = core[order]
    bo = b_all[order]
    to = t_all[order]
    so = srclo_all[order]
    do = dstlo_all[order]
    # phase 1: block-region row m = t*R + r at (p=m//CPB, c=m%CPB)
    m = to * R + r_sorted
    p1 = m // CPB
    c1 = m % CPB
    srcloc[co, 0, bo * SPB + c1 * 128 + p1] = so.astype(np.float32)
    # phase 2: tile stream position q = b*R + r at (p=q//CPT, j=q%CPT)
    q = bo * R + r_sorted
    p2 = q // CPT
    j2 = q % CPT
    dstloc[co, p2, to * CPT + j2] = do.astype(np.float32)

    dis_t = np.zeros((P_CORES, 128, NT), dtype=np.float32)
    dinv = np.zeros((P_CORES, 1, NPP), dtype=np.float32)
    for c in range(P_CORES):
        d = dis[c * NP:(c + 1) * NP]
        pad = np.zeros(NPP, dtype=np.float32)
        pad[:NP] = d
        dis_t[c] = pad.reshape(NT, 128).T
        ipad = np.zeros(NPP, dtype=np.float32)
        ipad[:NP] = 1.0 / d
        dinv[c, 0] = ipad

    return dict(dis=dis, R=R, SPB=SPB, CPB=CPB, CPT=CPT,
                srcloc=srcloc, dstloc=dstloc, dis_t=dis_t, dinv=dinv)


# ---------------------------------------------------------------------------
# Bass kernel
# ---------------------------------------------------------------------------

def _build_nc(prep):
    import concourse.bass as bass
    import concourse.bacc as bacc
    import concourse.tile as tile
    from concourse import mybir

    f32 = mybir.dt.float32
    AF = mybir.ActivationFunctionType
    ALU = mybir.AluOpType
    ds = bass.ds

    R, CPB, CPT, SPB = prep["R"], prep["CPB"], prep["CPT"], prep["SPB"]

    nc = bacc.Bacc(
        "TRN2", target_bir_lowering=False, debug=False,
        enable_asserts=False, num_devices=P_CORES,
    )

    H2 = SPB // 2
    xt_d = nc.dram_tensor("xt", [NPAD, DIN], f32, kind="ExternalInput")
    xown_d = nc.dram_tensor("xown", [NPP, DIN], f32, kind="ExternalInput")
    srcloc_d = nc.dram_tensor("srcloc", [1, NBM * SPB], f32,
                              kind="ExternalInput")
    ones1_d = nc.dram_tensor("ones1", [1, 128], f32, kind="ExternalInput")
    iotat_d = nc.dram_tensor("iotat", [128, H2], f32, kind="ExternalInput")
    dstloc_d = nc.dram_tensor("dstloc", [128, NT * CPT], f32,
                              kind="ExternalInput")
    dis_d = nc.dram_tensor("dis_t", [128, NT], f32, kind="ExternalInput")
    dinv_d = nc.dram_tensor("dinv", [1, NPP], f32, kind="ExternalInput")
    w1_d = nc.dram_tensor("w1", [DIN, DH], f32, kind="ExternalInput")
    w2_d = nc.dram_tensor("w2", [DH, DOUT], f32, kind="ExternalInput")
    b1_d = nc.dram_tensor("b1", [DH, 1], f32, kind="ExternalInput")
    b2_d = nc.dram_tensor("b2r", [1, DOUT], f32, kind="ExternalInput")
    iota_d = nc.dram_tensor("iota", [128, 128], f32, kind="ExternalInput")
    ident_d = nc.dram_tensor("ident", [128, 128], f32, kind="ExternalInput")
    out_d = nc.dram_tensor("outp", [NPP, DOUT], f32, kind="ExternalOutput")

    with tile.TileContext(nc) as tc:
        with (
            tc.tile_pool(name="const", bufs=1) as constp,
            tc.tile_pool(name="xb", bufs=3) as xpool,
            tc.tile_pool(name="stg", bufs=3) as stpool,
            tc.tile_pool(name="sl", bufs=3) as slpool,
            tc.tile_pool(name="sp", bufs=4) as spool,
            tc.tile_pool(name="msb", bufs=2) as mpool,
            tc.tile_pool(name="wk", bufs=2) as work,
            tc.tile_pool(name="p1", bufs=2, space="PSUM") as p1pool,
            tc.tile_pool(name="psel", bufs=1, space="PSUM") as pselp,
            tc.tile_pool(name="pagg", bufs=2, space="PSUM") as pagg,
            tc.tile_pool(name="ptr", bufs=1, space="PSUM") as ptr,
            tc.tile_pool(name="pg1", bufs=1, space="PSUM") as pg1,
            tc.tile_pool(name="pg2", bufs=1, space="PSUM") as pg2,
            tc.tile_pool(name="dram", bufs=1, space="DRAM") as dram,
        ):
            iota_sb = constp.tile([128, 128], f32)
            ident_sb = constp.tile([128, 128], f32)
            ones1_sb = constp.tile([1, 128], f32)
            iotat_sb = constp.tile([128, H2], f32)
            w1_sb = constp.tile([DIN, DH], f32)
            w2_sb = constp.tile([DH, DOUT], f32)
            b1_sb = constp.tile([DH, 1], f32)
            b2_sb = constp.tile([1, DOUT], f32)
            for sb, dr in [(iota_sb, iota_d), (ident_sb, ident_d),
                           (ones1_sb, ones1_d), (iotat_sb, iotat_d),
                           (w1_sb, w1_d), (w2_sb, w2_d),
                           (b1_sb, b1_d), (b2_sb, b2_d)]:
                nc.sync.dma_start(sb[:], dr[:])

            # M scratch lives across repeats; zero the pad-block regions
            # once (phase 2 reads them; dstloc=PAD keeps them out of sums,
            # but they must be finite).
            M_d = dram.tile([NBM, SPB * DIN], f32, tag="M", bufs=1)
            zt = work.tile([128, SPB * DIN // 128], f32, tag="zt")
            nc.gpsimd.memset(zt[:], 0.0)
            for b in range(NB, NBM):
                nc.sync.dma_start(M_d[b:b + 1, :], zt[:])

            for _rep in range(int(os.environ.get("GCN_REPEAT", "1"))):
                part = dram.tile([NPP, DOUT], f32, tag="part", bufs=2)
                table = dram.tile([NPAD, DOUT], f32, addr_space="Shared",
                                  tag="table", bufs=2)

                for lidx in range(2):
                    src_d = xt_d if lidx == 0 else table
                    own_d = xown_d if lidx == 0 else part
                    # ---- phase 1: route src blocks into bucket slots ----
                    with tc.For_i(0, NB) as i:
                        xb = xpool.tile([128, DIN], f32, tag="xb")
                        nc.sync.dma_start(xb[:], src_d[ds(i * 128, 128), :])
                        slst = slpool.tile([1, SPB], f32, tag="slst")
                        nc.sync.dma_start(
                            slst[:], srcloc_d[:, ds(i * SPB, SPB)])
                        # transposed one-hot: sel[s, j] = (s == srclo(slot j))
                        sel = spool.tile([128, SPB], f32, tag="sel")
                        for h in range(2):
                            pr = pselp.tile([128, H2], f32, tag="pr")
                            nc.tensor.matmul(
                                pr[:], ones1_sb[:],
                                slst[:, h * H2:(h + 1) * H2],
                                start=True, stop=True)
                            nc.vector.tensor_tensor(
                                sel[:, h * H2:(h + 1) * H2], iotat_sb[:],
                                pr[:], op=ALU.is_equal)
                        ps = p1pool.tile([128, CPB * DIN], f32, tag="ps")
                        for c in range(CPB):
                            nc.tensor.matmul(
                                ps[:, c * DIN:(c + 1) * DIN],
                                sel[:, c * 128:(c + 1) * 128], xb[:],
                                start=True, stop=True)
                        stage = stpool.tile([128, CPB * DIN], f32,
                                            tag="stage")
                        nc.scalar.activation(stage[:], ps[:], AF.Copy,
                                             bias=0.0)
                        nc.sync.dma_start(M_d[ds(i, 1), :], stage[:])
                    # ---- phase 2: per dst tile aggregate + layer tail ----
                    with tc.For_i(0, NT) as i:
                        msb = mpool.tile([128, CPT * DIN], f32, tag="msb")
                        nc.sync.dma_start(
                            msb[:],
                            M_d[:, ds(i * (R * DIN), R * DIN)].rearrange(
                                "b (r f) -> b r f", f=DIN))
                        xdg = xpool.tile([128, DIN], f32, tag="xdg")
                        nc.sync.dma_start(xdg[:], own_d[ds(i * 128, 128), :])
                        dlst = slpool.tile([128, CPT], f32, tag="dlst")
                        nc.sync.dma_start(
                            dlst[:], dstloc_d[:, ds(i * CPT, CPT)])
                        dcol = slpool.tile([128, 1], f32, tag="dcol")
                        nc.sync.dma_start(dcol[:], dis_d[:, ds(i, 1)])
                        # grouped one-hot build: 4 DVE ops cover CPT chunks
                        KG = CPT // 4
                        s_t = spool.tile([128, CPT * 128], f32, tag="s2",
                                         bufs=2)
                        for g in range(4):
                            i0, i1 = bass.broadcast_tensor_aps(
                                iota_sb[:].rearrange(
                                    "p (one j) -> p one j", one=1),
                                dlst[:, g * KG:(g + 1) * KG].rearrange(
                                    "p (k one) -> p k one", one=1))
                            nc.vector.tensor_tensor(
                                s_t[:, g * KG * 128:(g + 1) * KG * 128]
                                .rearrange("p (k j) -> p k j", j=128),
                                i0, i1, op=ALU.is_equal)
                        agg = pagg.tile([128, DIN], f32, tag="agg")
                        for j in range(CPT):
                            nc.tensor.matmul(
                                agg[:], s_t[:, j * 128:(j + 1) * 128],
                                msb[:, j * DIN:(j + 1) * DIN],
                                start=(j == 0), stop=False)
                        nc.tensor.matmul(agg[:], ident_sb[:], xdg[:],
                                         start=False, stop=(lidx == 0))
                        if lidx == 0:
                            a_sb = work.tile([128, DIN], f32, tag="a_sb")
                            nc.scalar.activation(a_sb[:], agg[:], AF.Copy,
                                                 bias=0.0,
                                                 scale=dcol[:, 0:1])
                            aT_p = ptr.tile([DIN, 128], f32, tag="aT")
                            nc.tensor.transpose(aT_p[:], a_sb[:], ident_sb[:])
                            aT_sb = work.tile([DIN, 128], f32, tag="aT_sb")
                            nc.scalar.activation(aT_sb[:], aT_p[:], AF.Copy,
                                                 bias=0.0)
                            x1_p = pg1.tile([DH, 128], f32, tag="x1")
                            nc.tensor.matmul(x1_p[:], w1_sb[:], aT_sb[:],
                                             start=True, stop=True)
                            x1_sb = work.tile([DH, 128], f32, tag="x1_sb")
                            nc.scalar.activation(x1_sb[:], x1_p[:], AF.Lrelu,
                                                 bias=b1_sb[:, 0:1],
                                                 alpha=NEG_SLOPE)
                            x2_p = pg2.tile([128, DOUT], f32, tag="x2")
                            nc.tensor.matmul(x2_p[:], x1_sb[:], w2_sb[:],
                                             start=True, stop=True)
                            x2_sb = work.tile([128, DOUT], f32, tag="x2_sb")
                            nc.scalar.activation(x2_sb[:], x2_p[:], AF.Copy,
                                                 bias=0.0,
                                                 scale=dcol[:, 0:1])
                            nc.sync.dma_start(
                                part[ds(i * 128, 128), :], x2_sb[:])
                        else:
                            div = slpool.tile([1, 128], f32, tag="div")
                            nc.sync.dma_start(
                                div[:], dinv_d[:, ds(i * 128, 128)])
                            nc.tensor.matmul(agg[:], div[:], b2_sb[:],
                                             start=False, stop=True)
                            o_sb = work.tile([128, DOUT], f32, tag="o_sb")
                            nc.scalar.activation(o_sb[:], agg[:], AF.Lrelu,
                                                 bias=0.0,
                                                 scale=dcol[:, 0:1],
                                                 alpha=NEG_SLOPE)
                            nc.sync.dma_start(
                                out_d[ds(i * 128, 128), :], o_sb[:])
                    if lidx == 0:
                        if os.environ.get("GCN_NOAG", "0") == "1":
                            # timing ablation: replace exchange with a local
                            # copy of this core's part (results are wrong)
                            nc.sync.dma_start(
                                table[0:NPP, :], part[:, :])
                        else:
                            nc.gpsimd.collective_compute(
                                "AllGather", mybir.AluOpType.bypass,
                                replica_groups=[list(range(P_CORES))],
                                ins=[part.opt()], outs=[table.opt()],
                            )

    nc.compile()
    return nc


def _make_in_maps(inputs, W1, b1, W2, b2, prep):
    dis = prep["dis"]
    xt = np.zeros((NPAD, DIN), dtype=np.float32)
    x32 = np.asarray(inputs, np.float32)
    for c in range(P_CORES):
        xt[c * NPP:c * NPP + NP] = (x32[c * NP:(c + 1) * NP]
                                    * dis[c * NP:(c + 1) * NP, None])
    iota = np.tile(np.arange(128, dtype=np.float32), (128, 1))
    ident = np.eye(128, dtype=np.float32)
    in_maps = []
    for c in range(P_CORES):
        in_maps.append({
            "xt": xt,
            "xown": xt[c * NPP:(c + 1) * NPP],
            "srcloc": prep["srcloc"][c],
            "dstloc": prep["dstloc"][c],
            "dis_t": prep["dis_t"][c],
            "dinv": prep["dinv"][c],
            "w1": np.asarray(W1, np.float32),
            "w2": np.asarray(W2, np.float32),
            "b1": np.asarray(b1, np.float32).reshape(DH, 1),
            "b2r": np.asarray(b2, np.float32).reshape(1, DOUT),
            "iota": iota,
            "ident": ident,
            "ones1": np.ones((1, 128), dtype=np.float32),
            "iotat": np.tile(
                np.arange(128, dtype=np.float32)[:, None],
                (1, prep["SPB"] // 2)),
        })
    return in_maps


_CACHE = {}


def kernel(inputs, edge_index, W1, b1, W2, b2, _trace=False, _results_box=None):
    from concourse.bass_utils import run_bass_kernel_spmd

    edge_index = np.asarray(edge_index)
    key = hashlib.sha1(edge_index.tobytes()).hexdigest()
    key += ":r%s:n%s" % (os.environ.get("GCN_REPEAT", "1"),
                         os.environ.get("GCN_NOAG", "0"))
    if key not in _CACHE:
        prep = _prep(edge_index)
        nc = _build_nc(prep)
        _CACHE[key] = (prep, nc)
    prep, nc = _CACHE[key]
    in_maps = _make_in_maps(inputs, W1, b1, W2, b2, prep)
    res = run_bass_kernel_spmd(
        nc, in_maps, core_ids=list(range(P_CORES)), trace=_trace,
    )
    if _results_box is not None:
        _results_box.append(res)
    out = np.concatenate(
        [res.results[c]["outp"][:NP] for c in range(P_CORES)], axis=0
    )
    return out.astype(np.float32)


# revision 24
# speedup vs baseline: 231.5675x; 14.2158x over previous
"""2-layer GCN (PyG GCNConv x2 + leaky_relu) on 8 Trainium2 NeuronCores.

v2 strategy (dst-partitioned, gather-free, For_i hardware loops):
  - Nodes partitioned 128-ALIGNED across 8 cores: core c owns padded ids
    [c*6272, c*6272+6250); padded table has NPAD=50176 rows (zeros in pads).
  - Normalization folded: table rows pre-scaled by dis[src]; dis[dst] applied
    post-aggregation (ACT scale). Self-loops NOT in the edge stream: handled
    by one identity matmul per dst tile reading the core's own rows
    (xown for layer 1, `part` for layer 2) — contributes dis_d^2 * x_d.
  - Edge routing has NO per-edge DMA. Two phases through a DRAM scratch M:
    Phase 1 (For_i over 392 src blocks): one-hot Sel (DVE is_equal vs iota)
      selects/duplicates rows of X_b [128,64] into bucket slots via PE
      matmul; 7 chunks/block -> stage [128,7*64] -> one contiguous DMA to
      M block region [896 rows, 64]. Block region row m holds slot
      (p=m//7, c=m%7); bucket (b,t) occupies rows m = t*16 + r (r<16).
    Phase 2 (For_i over 49 dst tiles): one strided DMA reads rows
      [16t,16t+16) of every block -> msb [128, 49*64] (slot q=b*16+r at
      partition q//49, col q%49); 49 one-hot scatter matmuls accumulate
      agg[128dst,64] in PSUM; + identity matmul (self loop) (+ rank-1
      disinv x b2 term closing layer 2's group).
  - Layer-1 tail per tile: ACT(dis) -> PE transpose -> W1 -> Lrelu+b1 ->
    W2 -> ACT(dis) -> part. One AllGather builds the layer-2 table.
  - Bucket capacity R = max bucket fill (>=16, 17 for this graph); M block
    slots padded to NBM=512 so CPT = 4R divides evenly for the strided DMA.
  - Routing path (tables, sel/scatter one-hots, M, exchange) runs in bf16
    (rel err ~1e-3, tol 2e-2); PSUM accumulation and the dense tail in f32.
  - One-hots are built in wide groups (broadcast tensor_tensor is_equal
    against iota) rather than per 128-slot chunk.

Self-contained: hardcodes shapes; compiles on first call keyed by edge hash.
"""

import os
import hashlib
import sys

import numpy as np

sys.path.insert(0, "/opt/trn_rl_repo")

# ---- problem constants ----
N, E = 50000, 800000
DIN, DH, DOUT = 64, 128, 64
P_CORES = 8
NP = N // P_CORES            # 6250 real nodes per core
NT = 49                      # dst tiles per core
NPP = NT * 128               # 6272 padded rows per core
NPAD = P_CORES * NPP         # 50176 padded table rows
NB = NPAD // 128             # 392 real src blocks (global)
NBM = 512                    # M-scratch block slots (pad => CPT = 4R exactly)
PAD = 200.0                  # one-hot miss value
NEG_SLOPE = 0.01


def _prep(edge_index: np.ndarray):
    src = np.asarray(edge_index[0], dtype=np.int64)
    dst = np.asarray(edge_index[1], dtype=np.int64)

    deg = (np.bincount(dst, minlength=N) + 1).astype(np.float32)
    dis = (1.0 / np.sqrt(deg)).astype(np.float32)

    pid_src = (src // NP) * NPP + (src % NP)      # padded id of src
    core = dst // NP
    tloc = (dst // NP) * 0 + (dst % NP)           # local dst 0..NP-1
    b_all = pid_src // 128                        # src block 0..NB-1
    t_all = tloc // 128                           # dst tile 0..NT-1
    srclo_all = pid_src % 128
    dstlo_all = tloc % 128

    # bucket ranks per (core, b, t)
    key = (core * NB + b_all) * NT + t_all
    order = np.argsort(key, kind="stable")
    ks = key[order]
    # rank within equal keys
    first = np.ones(len(ks), dtype=bool)
    first[1:] = ks[1:] != ks[:-1]
    starts = np.flatnonzero(first)
    run_id = np.cumsum(first) - 1
    r_sorted = np.arange(len(ks)) - starts[run_id]
    rmax = int(r_sorted.max()) + 1 if len(ks) else 1
    R = max(16, rmax)                             # bucket capacity
    SPB = ((NT * R + 127) // 128) * 128           # slots per block region
    CPB = SPB // 128                              # phase-1 chunks per block
    CPT = (NBM * R) // 128                        # phase-2 chunks per tile

    # srcloc: row vector of src-lo per slot, j = c*128 + p for slot at
    # physical row m = p*CPB + c (transposed one-hot built on device via
    # rank-1 PE broadcast + tensor_tensor is_equal)
    srcloc = np.full((P_CORES, 1, NBM * SPB), PAD, dtype=np.float32)
    dstloc = np.full((P_CORES, 128, NT * CPT), PAD, dtype=np.float32)

    co = core[order]
    bo = b_all[order]
    to = t_all[order]
    so = srclo_all[order]
    do = dstlo_all[order]
    # phase 1: block-region row m = t*R + r at (p=m//CPB, c=m%CPB)
    m = to * R + r_sorted
    p1 = m // CPB
    c1 = m % CPB
    srcloc[co, 0, bo * SPB + c1 * 128 + p1] = so.astype(np.float32)
    # phase 2: tile stream position q = b*R + r at (p=q//CPT, j=q%CPT)
    q = bo * R + r_sorted
    p2 = q // CPT
    j2 = q % CPT
    dstloc[co, p2, to * CPT + j2] = do.astype(np.float32)

    dis_t = np.zeros((P_CORES, 128, NT), dtype=np.float32)
    dinv = np.zeros((P_CORES, 1, NPP), dtype=np.float32)
    for c in range(P_CORES):
        d = dis[c * NP:(c + 1) * NP]
        pad = np.zeros(NPP, dtype=np.float32)
        pad[:NP] = d
        dis_t[c] = pad.reshape(NT, 128).T
        ipad = np.zeros(NPP, dtype=np.float32)
        ipad[:NP] = 1.0 / d
        dinv[c, 0] = ipad

    return dict(dis=dis, R=R, SPB=SPB, CPB=CPB, CPT=CPT,
                srcloc=srcloc, dstloc=dstloc, dis_t=dis_t, dinv=dinv)


# ---------------------------------------------------------------------------
# Bass kernel
# ---------------------------------------------------------------------------

def _build_nc(prep):
    import concourse.bass as bass
    import concourse.bacc as bacc
    import concourse.tile as tile
    from concourse import mybir

    f32 = mybir.dt.float32
    bf16 = mybir.dt.bfloat16
    AF = mybir.ActivationFunctionType
    ALU = mybir.AluOpType
    ds = bass.ds

    R, CPB, CPT, SPB = prep["R"], prep["CPB"], prep["CPT"], prep["SPB"]

    nc = bacc.Bacc(
        "TRN2", target_bir_lowering=False, debug=False,
        enable_asserts=False, num_devices=P_CORES,
    )

    H2 = SPB // 2
    xt_d = nc.dram_tensor("xt", [NPAD, DIN], bf16, kind="ExternalInput")
    xown_d = nc.dram_tensor("xown", [NPP, DIN], bf16, kind="ExternalInput")
    srcloc_d = nc.dram_tensor("srcloc", [1, NBM * SPB], f32,
                              kind="ExternalInput")
    ones1_d = nc.dram_tensor("ones1", [1, 128], f32, kind="ExternalInput")
    iotat_d = nc.dram_tensor("iotat", [128, H2], f32, kind="ExternalInput")
    dstloc_d = nc.dram_tensor("dstloc", [128, NT * CPT], f32,
                              kind="ExternalInput")
    dis_d = nc.dram_tensor("dis_t", [128, NT], f32, kind="ExternalInput")
    dinv_d = nc.dram_tensor("dinv", [1, NPP], f32, kind="ExternalInput")
    w1_d = nc.dram_tensor("w1", [DIN, DH], f32, kind="ExternalInput")
    w2_d = nc.dram_tensor("w2", [DH, DOUT], f32, kind="ExternalInput")
    b1_d = nc.dram_tensor("b1", [DH, 1], f32, kind="ExternalInput")
    b2_d = nc.dram_tensor("b2r", [1, DOUT], f32, kind="ExternalInput")
    iota_d = nc.dram_tensor("iota", [128, 128], f32, kind="ExternalInput")
    ident_d = nc.dram_tensor("ident", [128, 128], f32, kind="ExternalInput")
    identb_d = nc.dram_tensor("identb", [128, 128], bf16, kind="ExternalInput")
    out_d = nc.dram_tensor("outp", [NPP, DOUT], f32, kind="ExternalOutput")

    with tile.TileContext(nc) as tc:
        with (
            tc.tile_pool(name="const", bufs=1) as constp,
            tc.tile_pool(name="xb", bufs=3) as xpool,
            tc.tile_pool(name="stg", bufs=3) as stpool,
            tc.tile_pool(name="sl", bufs=3) as slpool,
            tc.tile_pool(name="sp", bufs=4) as spool,
            tc.tile_pool(name="msb", bufs=2) as mpool,
            tc.tile_pool(name="wk", bufs=2) as work,
            tc.tile_pool(name="p1", bufs=2, space="PSUM") as p1pool,
            tc.tile_pool(name="psel", bufs=1, space="PSUM") as pselp,
            tc.tile_pool(name="pagg", bufs=2, space="PSUM") as pagg,
            tc.tile_pool(name="ptr", bufs=1, space="PSUM") as ptr,
            tc.tile_pool(name="pg1", bufs=1, space="PSUM") as pg1,
            tc.tile_pool(name="pg2", bufs=1, space="PSUM") as pg2,
            tc.tile_pool(name="dram", bufs=1, space="DRAM") as dram,
        ):
            iota_sb = constp.tile([128, 128], f32)
            ident_sb = constp.tile([128, 128], f32)
            identb_sb = constp.tile([128, 128], bf16)
            ones1_sb = constp.tile([1, 128], f32)
            iotat_sb = constp.tile([128, H2], f32)
            w1_sb = constp.tile([DIN, DH], f32)
            w2_sb = constp.tile([DH, DOUT], f32)
            b1_sb = constp.tile([DH, 1], f32)
            b2_sb = constp.tile([1, DOUT], f32)
            for sb, dr in [(iota_sb, iota_d), (ident_sb, ident_d),
                           (identb_sb, identb_d),
                           (ones1_sb, ones1_d), (iotat_sb, iotat_d),
                           (w1_sb, w1_d), (w2_sb, w2_d),
                           (b1_sb, b1_d), (b2_sb, b2_d)]:
                nc.sync.dma_start(sb[:], dr[:])

            # M scratch lives across repeats; zero the pad-block regions
            # once (phase 2 reads them; dstloc=PAD keeps them out of sums,
            # but they must be finite).
            M_d = dram.tile([NBM, SPB * DIN], bf16, tag="M", bufs=1)
            zt = work.tile([128, SPB * DIN // 128], bf16, tag="zt")
            nc.gpsimd.memset(zt[:], 0.0)
            for b in range(NB, NBM):
                nc.sync.dma_start(M_d[b:b + 1, :], zt[:])

            for _rep in range(int(os.environ.get("GCN_REPEAT", "1"))):
                part = dram.tile([NPP, DOUT], bf16, tag="part", bufs=2)
                table = dram.tile([NPAD, DOUT], bf16, addr_space="Shared",
                                  tag="table", bufs=2)

                for lidx in range(2):
                    src_d = xt_d if lidx == 0 else table
                    own_d = xown_d if lidx == 0 else part
                    # ---- phase 1: route src blocks into bucket slots ----
                    with tc.For_i(0, NB) as i:
                        xb = xpool.tile([128, DIN], bf16, tag="xb")
                        nc.sync.dma_start(xb[:], src_d[ds(i * 128, 128), :])
                        slst = slpool.tile([1, SPB], f32, tag="slst")
                        nc.sync.dma_start(
                            slst[:], srcloc_d[:, ds(i * SPB, SPB)])
                        # transposed one-hot: sel[s, j] = (s == srclo(slot j))
                        sel = spool.tile([128, SPB], bf16, tag="sel")
                        for h in range(2):
                            pr = pselp.tile([128, H2], f32, tag="pr")
                            nc.tensor.matmul(
                                pr[:], ones1_sb[:],
                                slst[:, h * H2:(h + 1) * H2],
                                start=True, stop=True)
                            nc.vector.tensor_tensor(
                                sel[:, h * H2:(h + 1) * H2], iotat_sb[:],
                                pr[:], op=ALU.is_equal)
                        ps = p1pool.tile([128, CPB * DIN], f32, tag="ps")
                        for c in range(CPB):
                            nc.tensor.matmul(
                                ps[:, c * DIN:(c + 1) * DIN],
                                sel[:, c * 128:(c + 1) * 128], xb[:],
                                start=True, stop=True)
                        stage = stpool.tile([128, CPB * DIN], bf16,
                                            tag="stage")
                        nc.scalar.activation(stage[:], ps[:], AF.Copy,
                                             bias=0.0)
                        nc.sync.dma_start(M_d[ds(i, 1), :], stage[:])
                    # ---- phase 2: per dst tile aggregate + layer tail ----
                    with tc.For_i(0, NT) as i:
                        msb = mpool.tile([128, CPT * DIN], bf16, tag="msb")
                        nc.sync.dma_start(
                            msb[:],
                            M_d[:, ds(i * (R * DIN), R * DIN)].rearrange(
                                "b (r f) -> b r f", f=DIN))
                        xdg = xpool.tile([128, DIN], bf16, tag="xdg")
                        nc.sync.dma_start(xdg[:], own_d[ds(i * 128, 128), :])
                        dlst = slpool.tile([128, CPT], f32, tag="dlst")
                        nc.sync.dma_start(
                            dlst[:], dstloc_d[:, ds(i * CPT, CPT)])
                        dcol = slpool.tile([128, 1], f32, tag="dcol")
                        nc.sync.dma_start(dcol[:], dis_d[:, ds(i, 1)])
                        # grouped one-hot build: 4 DVE ops cover CPT chunks
                        KG = CPT // 4
                        s_t = spool.tile([128, CPT * 128], bf16, tag="s2",
                                         bufs=2)
                        for g in range(4):
                            i0, i1 = bass.broadcast_tensor_aps(
                                iota_sb[:].rearrange(
                                    "p (one j) -> p one j", one=1),
                                dlst[:, g * KG:(g + 1) * KG].rearrange(
                                    "p (k one) -> p k one", one=1))
                            nc.vector.tensor_tensor(
                                s_t[:, g * KG * 128:(g + 1) * KG * 128]
                                .rearrange("p (k j) -> p k j", j=128),
                                i0, i1, op=ALU.is_equal)
                        agg = pagg.tile([128, DIN], f32, tag="agg")
                        for j in range(CPT):
                            nc.tensor.matmul(
                                agg[:], s_t[:, j * 128:(j + 1) * 128],
                                msb[:, j * DIN:(j + 1) * DIN],
                                start=(j == 0), stop=False)
                        nc.tensor.matmul(agg[:], identb_sb[:], xdg[:],
                                         start=False, stop=(lidx == 0))
                        if lidx == 0:
                            a_sb = work.tile([128, DIN], f32, tag="a_sb")
                            nc.scalar.activation(a_sb[:], agg[:], AF.Copy,
                                                 bias=0.0,
                                                 scale=dcol[:, 0:1])
                            aT_p = ptr.tile([DIN, 128], f32, tag="aT")
                            nc.tensor.transpose(aT_p[:], a_sb[:], ident_sb[:])
                            aT_sb = work.tile([DIN, 128], f32, tag="aT_sb")
                            nc.scalar.activation(aT_sb[:], aT_p[:], AF.Copy,
                                                 bias=0.0)
                            x1_p = pg1.tile([DH, 128], f32, tag="x1")
                            nc.tensor.matmul(x1_p[:], w1_sb[:], aT_sb[:],
                                             start=True, stop=True)
                            x1_sb = work.tile([DH, 128], f32, tag="x1_sb")
                            nc.scalar.activation(x1_sb[:], x1_p[:], AF.Lrelu,
                                                 bias=b1_sb[:, 0:1],
                                                 alpha=NEG_SLOPE)
                            x2_p = pg2.tile([128, DOUT], f32, tag="x2")
                            nc.tensor.matmul(x2_p[:], x1_sb[:], w2_sb[:],
                                             start=True, stop=True)
                            x2_sb = work.tile([128, DOUT], bf16, tag="x2_sb")
                            nc.scalar.activation(x2_sb[:], x2_p[:], AF.Copy,
                                                 bias=0.0,
                                                 scale=dcol[:, 0:1])
                            nc.sync.dma_start(
                                part[ds(i * 128, 128), :], x2_sb[:])
                        else:
                            div = slpool.tile([1, 128], f32, tag="div")
                            nc.sync.dma_start(
                                div[:], dinv_d[:, ds(i * 128, 128)])
                            nc.tensor.matmul(agg[:], div[:], b2_sb[:],
                                             start=False, stop=True)
                            o_sb = work.tile([128, DOUT], f32, tag="o_sb")
                            nc.scalar.activation(o_sb[:], agg[:], AF.Lrelu,
                                                 bias=0.0,
                                                 scale=dcol[:, 0:1],
                                                 alpha=NEG_SLOPE)
                            nc.sync.dma_start(
                                out_d[ds(i * 128, 128), :], o_sb[:])
                    if lidx == 0:
                        if os.environ.get("GCN_NOAG", "0") == "1":
                            # timing ablation: replace exchange with a local
                            # copy of this core's part (results are wrong)
                            nc.sync.dma_start(
                                table[0:NPP, :], part[:, :])
                        else:
                            nc.gpsimd.collective_compute(
                                "AllGather", mybir.AluOpType.bypass,
                                replica_groups=[list(range(P_CORES))],
                                ins=[part.opt()], outs=[table.opt()],
                            )

    nc.compile()
    return nc


def _make_in_maps(inputs, W1, b1, W2, b2, prep):
    import ml_dtypes
    dis = prep["dis"]
    xt = np.zeros((NPAD, DIN), dtype=np.float32)
    x32 = np.asarray(inputs, np.float32)
    for c in range(P_CORES):
        xt[c * NPP:c * NPP + NP] = (x32[c * NP:(c + 1) * NP]
                                    * dis[c * NP:(c + 1) * NP, None])
    xt = xt.astype(ml_dtypes.bfloat16)
    iota = np.tile(np.arange(128, dtype=np.float32), (128, 1))
    ident = np.eye(128, dtype=np.float32)
    in_maps = []
    for c in range(P_CORES):
        in_maps.append({
            "xt": xt,
            "xown": xt[c * NPP:(c + 1) * NPP],
            "srcloc": prep["srcloc"][c],
            "dstloc": prep["dstloc"][c],
            "dis_t": prep["dis_t"][c],
            "dinv": prep["dinv"][c],
            "w1": np.asarray(W1, np.float32),
            "w2": np.asarray(W2, np.float32),
            "b1": np.asarray(b1, np.float32).reshape(DH, 1),
            "b2r": np.asarray(b2, np.float32).reshape(1, DOUT),
            "iota": iota,
            "ident": ident,
            "identb": ident.astype(ml_dtypes.bfloat16),
            "ones1": np.ones((1, 128), dtype=np.float32),
            "iotat": np.tile(
                np.arange(128, dtype=np.float32)[:, None],
                (1, prep["SPB"] // 2)),
        })
    return in_maps


_CACHE = {}


def kernel(inputs, edge_index, W1, b1, W2, b2, _trace=False, _results_box=None):
    from concourse.bass_utils import run_bass_kernel_spmd

    edge_index = np.asarray(edge_index)
    key = hashlib.sha1(edge_index.tobytes()).hexdigest()
    key += ":r%s:n%s" % (os.environ.get("GCN_REPEAT", "1"),
                         os.environ.get("GCN_NOAG", "0"))
    if key not in _CACHE:
        prep = _prep(edge_index)
        nc = _build_nc(prep)
        _CACHE[key] = (prep, nc)
    prep, nc = _CACHE[key]
    in_maps = _make_in_maps(inputs, W1, b1, W2, b2, prep)
    res = run_bass_kernel_spmd(
        nc, in_maps, core_ids=list(range(P_CORES)), trace=_trace,
    )
    if _results_box is not None:
        _results_box.append(res)
    out = np.concatenate(
        [res.results[c]["outp"][:NP] for c in range(P_CORES)], axis=0
    )
    return out.astype(np.float32)
